# revision 40
# baseline (speedup 1.0000x reference)
"""MoD wrapper (router -> top-k -> gather -> GELU MLP -> weighted scatter-add)
on 8 Trainium2 NeuronCores.

Sharding: data-parallel over batch (4 sequences) x 2-way split of each
sequence's K=2048 selected tokens -> 8 cores, 1024 tokens each. Every core
holds the full FFN weights and computes
    y[t, :] = gate[t] * gelu_tanh(x[t, :] @ w1 + b1) @ w2
for its 1024 tokens. Routing (scores / top-k / sigmoid) runs through the
same jax ops the reference uses, so token selection matches the reference
bit-for-bit; gather and the final scatter-add into the residual stream are
host-side numpy (b2 is folded into the scatter).

Both GEMMs run in fp8 e4m3 (DoubleRow, 2x PE throughput). The weights sit
in e4m3's subnormal range, so they are pre-scaled by powers of two
(w1*64, w2*128) before quantization; the descales are exact folds — 1/64
into the gelu activation's input scale, 1/128 into the host-side gate.

Phase 2 computes the transposed product yT[d,t] = sum_f w2[f,d]*hT[f,t] so
that w2 is the cached stationary operand (one 16.8 MB pass instead of two)
and the SBUF-resident hT needs no DMA at all; the per-token gate (and the
1/128 descale) then moves to the host scatter, which is linear in y.
"""

import sys
import types

import numpy as np
import ml_dtypes

# bass_utils' trace path does `from antenv.axon_hooks import ...`; some
# images ship an antenv without that module (boot degrades silently but the
# import in bass_utils would crash). Register a no-op stand-in so trace=True
# degrades to "no profile" instead of raising.
try:
    import antenv.axon_hooks  # noqa: F401
except Exception:
    import antenv

    _hooks = types.ModuleType("antenv.axon_hooks")
    _hooks._hook = None
    _hooks.set_axon_ntff_profile_hook = \
        lambda h: setattr(_hooks, "_hook", h)
    _hooks.get_axon_ntff_profile_hook = \
        lambda: getattr(_hooks, "_hook", None)
    sys.modules["antenv.axon_hooks"] = _hooks
    antenv.axon_hooks = _hooks
    try:
        # Same registration trn_agent_boot.boot() would have done had the
        # module existed at interpreter start.
        from trn_agent_boot.trn_boot import _ntff_profile_via_ctypes

        _hook = _ntff_profile_via_ctypes("/opt/axon/libaxon_pjrt.so")
        if _hook is not None:
            _hooks.set_axon_ntff_profile_hook(_hook)
    except Exception:
        pass

import concourse.bacc as bacc
import concourse.bass as bass
import concourse.mybir as mybir
import concourse.tile as tile
from concourse.bass import ts
from concourse.bass_utils import run_bass_kernel_spmd
from concourse.kernels.tile_matmul import (
    ShapeInfo,
    composable_matmul_tile_kernel,
    dma_from_dram_kxm,
    dma_from_dram_kxn,
    dma_to_dram_mxn,
)

B, S, D, DFF = 4, 4096, 2048, 8192
K = 2048          # selected tokens per sequence
N_CORES = 8
TPC = (B * K) // N_CORES  # tokens per core = 1024

BF16 = mybir.dt.bfloat16
F32 = mybir.dt.float32
FP8 = mybir.dt.float8e4
P = 128

W1_SCALE = 64.0    # lifts w1 (std ~0.022) out of e4m3 subnormals
W2_SCALE = 128.0   # same for w2 (std ~0.011)


def _build_nc():
    nc = bacc.Bacc("TRN2", target_bir_lowering=False, debug=False,
                   num_devices=N_CORES)

    xT_ap = nc.dram_tensor("xT", [D, TPC], FP8, kind="ExternalInput").ap()
    w1_ap = nc.dram_tensor("w1", [D, DFF], FP8, kind="ExternalInput").ap()
    w2_ap = nc.dram_tensor("w2", [DFF, D], FP8, kind="ExternalInput").ap()
    b1_ap = nc.dram_tensor("b1v", [P, DFF // P], F32, kind="ExternalInput").ap()
    yT_ap = nc.dram_tensor("yT", [D, TPC], BF16, kind="ExternalOutput").ap()

    with tile.TileContext(nc) as tc:
        with (
            tc.tile_pool(name="const", bufs=1) as const_pool,
            tc.tile_pool(name="hT", bufs=1) as hT_pool,
            tc.tile_pool(name="kxm1", bufs=9) as kxm1_pool,
            tc.tile_pool(name="kxn1", bufs=9) as kxn1_pool,
            tc.tile_pool(name="kxm2", bufs=20) as kxm2_pool,
        ):
            b1_sb = const_pool.tile([P, DFF // P], F32)
            nc.gpsimd.dma_start(b1_sb[:], b1_ap[:])

            # PE warmup: dependency-free matmuls that run during the initial
            # weight-DMA fill so the HAM clock gate reaches 8/8 before the
            # first real matmul (saves the ~3.4us half-rate ramp).
            warm_sb = const_pool.tile([P, 64], FP8)
            nc.vector.memset(warm_sb[:], 0)
            with tc.tile_pool(name="warm_ps", bufs=1,
                              space="PSUM") as warm_pool:
                warm_ps = warm_pool.tile([P, 64], F32)
                for _ in range(96):
                    nc.tensor.matmul(warm_ps[:64, :], warm_sb[:], warm_sb[:],
                                     start=True, stop=True)

            # Intermediate hT[f, t] = gelu(w1.T @ x.T + b1), kept in SBUF
            # as the kxn operand of the second matmul. [128, 64, 1024] fp8.
            hT_cache = hT_pool.tile([P, DFF // P, TPC], FP8)

            # The first matmul waits on the (k0, m0) w1 tile and the (k0, n0)
            # xT tile. Load each as two half-tile DMAs on the two HWDGE
            # engines (separate queue sets) so they land in ~half the time.
            # dma_start issue itself costs ~600ns of sequencer time, so only
            # these two critical tiles get the split treatment.
            w1_t = w1_ap.rearrange("(po pi) f -> pi po f", pi=P)
            xT_t = xT_ap.rearrange("(po pi) f -> pi po f", pi=P)
            w1_first = const_pool.tile([P, 4, 512], FP8, name="w1_first")
            x_first = const_pool.tile([P, 4, 512], FP8, name="x_first")
            nc.sync.dma_start(w1_first[:, 0:2], w1_t[:, 0:2, 0:512])
            nc.scalar.dma_start(w1_first[:, 2:4], w1_t[:, 2:4, 0:512])
            nc.sync.dma_start(x_first[:, 0:2], xT_t[:, 0:2, 0:512])
            nc.scalar.dma_start(x_first[:, 2:4], xT_t[:, 2:4, 0:512])

            # ---- phase 1: hT = gelu(w1.T @ xT + b1) ----
            kxm1_dma, kxm1_shape = dma_from_dram_kxm(kxm1_pool, w1_ap)
            kxn1_dma, kxn1_shape = dma_from_dram_kxn(kxn1_pool, xT_ap)

            def kxm1_producer(nc_, md):
                if md.k_tile_idx == 0 and md.m_tile_idx == 0:
                    return w1_first[:]
                return kxm1_dma(nc_, md)

            def kxn1_producer(nc_, md):
                if md.k_tile_idx == 0 and md.n_tile_idx == 0:
                    return x_first[:]
                return kxn1_dma(nc_, md)

            def hT_slice_producer(nc_, md):
                return hT_cache[:, ts(md.m_tile_idx, md.m_subtiles), md.n_slice]

            def gelu_reducer(nc_, psum, sbuf, md):
                f_outer = md.m_tile_idx * md.m_subtiles + md.m_subtile_idx
                nc_.scalar.activation(
                    sbuf,
                    psum,
                    mybir.ActivationFunctionType.Gelu_apprx_tanh,
                    bias=b1_sb[:, f_outer:f_outer + 1],
                    scale=1.0 / W1_SCALE,
                )

            composable_matmul_tile_kernel(
                tc,
                kxm_shape=kxm1_shape,
                kxn_shape=kxn1_shape,
                output_type=None,
                kxm_producer=kxm1_producer,
                kxn_producer=kxn1_producer,
                mxn_consumer=lambda nc_, t, md: None,
                mxn_subtile_reducer=gelu_reducer,
                mxn_subtile_producer=hT_slice_producer,
                cache_tiles=True,
            )

            # ---- phase 2: yT[d, t] = sum_f w2[f, d] * hT[f, t] ----
            # w2 is the cached kxm (one DMA pass); hT is the SBUF-resident
            # kxn (no DMA). Gate + 1/128 descale applied on the host.
            kxm2_producer, kxm2_shape = dma_from_dram_kxm(kxm2_pool, w2_ap)
            kxn2_shape = ShapeInfo(pdims=((P, DFF // P),), fdims=(TPC,))

            def hT_kxn_producer(nc_, md):
                return hT_cache[:, ts(md.k_tile_idx, md.k_subtiles),
                                ts(md.n_tile_idx, md.n_tile)]

            # Write each output group as one DMA per m-subtile, alternating
            # HWDGE engines, so the final (exposed) write lands ~4x faster
            # than a single 512 KB transfer on one queue.
            yT_t = yT_ap.rearrange("(po pi) f -> pi po f", pi=P)

            def yT_consumer(nc_, mxn_tile, md):
                for i in range(md.m_subtiles):
                    eng = (nc_.sync, nc_.scalar)[i % 2]
                    eng.dma_start(
                        yT_t[:, md.m_tile_idx * md.m_subtiles + i,
                             ts(md.n_tile_idx, md.n_tile)],
                        mxn_tile[:, i])

            composable_matmul_tile_kernel(
                tc,
                kxm_shape=kxm2_shape,
                kxn_shape=kxn2_shape,
                output_type=BF16,
                kxm_producer=kxm2_producer,
                kxn_producer=hT_kxn_producer,
                mxn_consumer=yT_consumer,
                cache_tiles=True,
                psum_n_bufs=2,
            )

    nc.compile()
    return nc


_NC = None


def _routing(hidden_states, router_weight, router_bias):
    """Same ops/backend as the reference => bit-identical selection."""
    import jax
    import jax.numpy as jnp
    scores = jnp.einsum('bsd,d->bs', hidden_states, router_weight) \
        + router_bias[0]
    top_scores, indices = jax.lax.top_k(scores, K)
    weights = jax.nn.sigmoid(top_scores)
    return np.asarray(indices), np.asarray(weights)


def _run(hidden_states, router_weight, router_bias, w1, b1, w2, b2,
         trace=False):
    global _NC
    hidden_states = np.asarray(hidden_states, dtype=np.float32)
    router_weight = np.asarray(router_weight, dtype=np.float32)
    router_bias = np.asarray(router_bias, dtype=np.float32)
    w1 = np.asarray(w1, dtype=np.float32)
    b1 = np.asarray(b1, dtype=np.float32)
    w2 = np.asarray(w2, dtype=np.float32)
    b2 = np.asarray(b2, dtype=np.float32)

    indices, weights = _routing(hidden_states, router_weight, router_bias)

    if _NC is None:
        _NC = _build_nc()

    w1_q = (w1 * np.float32(W1_SCALE)).astype(ml_dtypes.float8_e4m3fn)
    w2_q = (w2 * np.float32(W2_SCALE)).astype(ml_dtypes.float8_e4m3fn)
    b1v = np.ascontiguousarray(b1.reshape(DFF // P, P).T)

    in_maps = []
    core_idx = []  # (b, idx_slice) per core
    for c in range(N_CORES):
        b, h = divmod(c, 2)
        idx_c = indices[b, h * TPC:(h + 1) * TPC]
        gate_c = weights[b, h * TPC:(h + 1) * TPC]
        xT = hidden_states[b, idx_c].T.astype(ml_dtypes.float8_e4m3fn)
        in_maps.append({
            "xT": xT,
            "w1": w1_q,
            "w2": w2_q,
            "b1v": b1v,
        })
        core_idx.append((b, idx_c, gate_c))

    res = run_bass_kernel_spmd(_NC, in_maps, core_ids=list(range(N_CORES)),
                               trace=trace)

    out = hidden_states.copy().reshape(B * S, D)
    b2_nonzero = bool(np.any(b2))
    for c in range(N_CORES):
        b, idx_c, gate_c = core_idx[c]
        yT = res.results[c]["yT"].astype(np.float32)  # [D, TPC] bf16
        g = gate_c * np.float32(1.0 / W2_SCALE)
        y = yT.T * g[:, None]
        if b2_nonzero:
            y += gate_c[:, None] * b2[None, :]
        out[b * S + idx_c] += y
    return out.reshape(B, S, D), res


def kernel(**inputs):
    return _run(**inputs)[0]



# revision 43
# speedup vs baseline: 1.0050x; 1.0050x over previous
"""MoD wrapper (router -> top-k -> gather -> GELU MLP -> weighted scatter-add)
on 8 Trainium2 NeuronCores.

Sharding: data-parallel over batch (4 sequences) x 2-way split of each
sequence's K=2048 selected tokens -> 8 cores, 1024 tokens each. Every core
holds the full FFN weights and computes
    y[t, :] = gate[t] * gelu_tanh(x[t, :] @ w1 + b1) @ w2
for its 1024 tokens. Routing (scores / top-k / sigmoid) runs through the
same jax ops the reference uses, so token selection matches the reference
bit-for-bit; gather and the final scatter-add into the residual stream are
host-side numpy (b2 is folded into the scatter).

Both GEMMs run in fp8 e4m3 (DoubleRow, 2x PE throughput). The weights sit
in e4m3's subnormal range, so they are pre-scaled by powers of two
(w1*64, w2*128) before quantization; the descales are exact folds — 1/64
into the gelu activation's input scale, 1/128 into the host-side gate.

Phase 2 computes the transposed product yT[d,t] = sum_f w2[f,d]*hT[f,t] so
that w2 is the cached stationary operand (one 16.8 MB pass instead of two)
and the SBUF-resident hT needs no DMA at all; the per-token gate (and the
1/128 descale) then moves to the host scatter, which is linear in y.
"""

import sys
import types

import numpy as np
import ml_dtypes

# bass_utils' trace path does `from antenv.axon_hooks import ...`; some
# images ship an antenv without that module (boot degrades silently but the
# import in bass_utils would crash). Register a no-op stand-in so trace=True
# degrades to "no profile" instead of raising.
try:
    import antenv.axon_hooks  # noqa: F401
except Exception:
    import antenv

    _hooks = types.ModuleType("antenv.axon_hooks")
    _hooks._hook = None
    _hooks.set_axon_ntff_profile_hook = \
        lambda h: setattr(_hooks, "_hook", h)
    _hooks.get_axon_ntff_profile_hook = \
        lambda: getattr(_hooks, "_hook", None)
    sys.modules["antenv.axon_hooks"] = _hooks
    antenv.axon_hooks = _hooks
    try:
        # Same registration trn_agent_boot.boot() would have done had the
        # module existed at interpreter start.
        from trn_agent_boot.trn_boot import _ntff_profile_via_ctypes

        _hook = _ntff_profile_via_ctypes("/opt/axon/libaxon_pjrt.so")
        if _hook is not None:
            _hooks.set_axon_ntff_profile_hook(_hook)
    except Exception:
        pass

import concourse.bacc as bacc
import concourse.bass as bass
import concourse.mybir as mybir
import concourse.tile as tile
from concourse.bass import ts
from concourse.bass_utils import run_bass_kernel_spmd
from concourse.kernels.tile_matmul import (
    ShapeInfo,
    composable_matmul_tile_kernel,
    dma_from_dram_kxm,
    dma_from_dram_kxn,
    dma_to_dram_mxn,
)

B, S, D, DFF = 4, 4096, 2048, 8192
K = 2048          # selected tokens per sequence
N_CORES = 8
TPC = (B * K) // N_CORES  # tokens per core = 1024

BF16 = mybir.dt.bfloat16
F32 = mybir.dt.float32
FP8 = mybir.dt.float8e4
P = 128

W1_SCALE = 64.0    # lifts w1 (std ~0.022) out of e4m3 subnormals
W2_SCALE = 128.0   # same for w2 (std ~0.011)


def _build_nc():
    nc = bacc.Bacc("TRN2", target_bir_lowering=False, debug=False,
                   num_devices=N_CORES)

    xT_ap = nc.dram_tensor("xT", [D, TPC], FP8, kind="ExternalInput").ap()
    w1_ap = nc.dram_tensor("w1", [D, DFF], FP8, kind="ExternalInput").ap()
    w2_ap = nc.dram_tensor("w2", [DFF, D], FP8, kind="ExternalInput").ap()
    b1_ap = nc.dram_tensor("b1v", [P, DFF // P], F32, kind="ExternalInput").ap()
    yT_ap = nc.dram_tensor("yT", [D, TPC], BF16, kind="ExternalOutput").ap()

    with tile.TileContext(nc) as tc:
        with (
            tc.tile_pool(name="const", bufs=1) as const_pool,
            tc.tile_pool(name="hT", bufs=1) as hT_pool,
            tc.tile_pool(name="kxm1", bufs=9) as kxm1_pool,
            tc.tile_pool(name="kxn1", bufs=9) as kxn1_pool,
            tc.tile_pool(name="kxm2", bufs=20) as kxm2_pool,
        ):
            b1_sb = const_pool.tile([P, DFF // P], F32)
            nc.gpsimd.dma_start(b1_sb[:], b1_ap[:])

            # PE warmup: dependency-free matmuls that run during the initial
            # weight-DMA fill so the HAM clock gate reaches 8/8 before the
            # first real matmul (saves the ~3.4us half-rate ramp).
            warm_sb = const_pool.tile([P, 64], FP8)
            nc.vector.memset(warm_sb[:], 0)
            with tc.tile_pool(name="warm_ps", bufs=1,
                              space="PSUM") as warm_pool:
                warm_ps = warm_pool.tile([P, 64], F32)
                for _ in range(96):
                    nc.tensor.matmul(warm_ps[:64, :], warm_sb[:], warm_sb[:],
                                     start=True, stop=True)

            # Intermediate hT[f, t] = gelu(w1.T @ x.T + b1), kept in SBUF
            # as the kxn operand of the second matmul. Split into one tile
            # per phase-2 k-tile (= per phase-1 m-tile) so phase 2's reads
            # only depend on the writes of their own 512-row f-block —
            # with a single big tile, phase 2's first matmul WAR-waits on
            # phase 1's very last gelu drain (~1.2us at the boundary).
            hT_tiles = [hT_pool.tile([P, 4, TPC], FP8, name=f"hT_{m}")
                        for m in range(DFF // P // 4)]

            # The first matmul waits on the (k0, m0) w1 tile and the (k0, n0)
            # xT tile. Load each as two half-tile DMAs on the two HWDGE
            # engines (separate queue sets) so they land in ~half the time.
            # dma_start issue itself costs ~600ns of sequencer time, so only
            # these two critical tiles get the split treatment.
            w1_t = w1_ap.rearrange("(po pi) f -> pi po f", pi=P)
            xT_t = xT_ap.rearrange("(po pi) f -> pi po f", pi=P)
            w1_first = const_pool.tile([P, 4, 512], FP8, name="w1_first")
            x_first = const_pool.tile([P, 4, 512], FP8, name="x_first")
            nc.sync.dma_start(w1_first[:, 0:2], w1_t[:, 0:2, 0:512])
            nc.scalar.dma_start(w1_first[:, 2:4], w1_t[:, 2:4, 0:512])
            nc.sync.dma_start(x_first[:, 0:2], xT_t[:, 0:2, 0:512])
            nc.scalar.dma_start(x_first[:, 2:4], xT_t[:, 2:4, 0:512])

            # ---- phase 1: hT = gelu(w1.T @ xT + b1) ----
            kxm1_dma, kxm1_shape = dma_from_dram_kxm(kxm1_pool, w1_ap)
            kxn1_dma, kxn1_shape = dma_from_dram_kxn(kxn1_pool, xT_ap)

            def kxm1_producer(nc_, md):
                if md.k_tile_idx == 0 and md.m_tile_idx == 0:
                    return w1_first[:]
                return kxm1_dma(nc_, md)

            def kxn1_producer(nc_, md):
                if md.k_tile_idx == 0 and md.n_tile_idx == 0:
                    return x_first[:]
                return kxn1_dma(nc_, md)

            def hT_slice_producer(nc_, md):
                assert md.m_subtiles == 4
                return hT_tiles[md.m_tile_idx][:, :, md.n_slice]

            def gelu_reducer(nc_, psum, sbuf, md):
                f_outer = md.m_tile_idx * md.m_subtiles + md.m_subtile_idx
                nc_.scalar.activation(
                    sbuf,
                    psum,
                    mybir.ActivationFunctionType.Gelu_apprx_tanh,
                    bias=b1_sb[:, f_outer:f_outer + 1],
                    scale=1.0 / W1_SCALE,
                )

            composable_matmul_tile_kernel(
                tc,
                kxm_shape=kxm1_shape,
                kxn_shape=kxn1_shape,
                output_type=None,
                kxm_producer=kxm1_producer,
                kxn_producer=kxn1_producer,
                mxn_consumer=lambda nc_, t, md: None,
                mxn_subtile_reducer=gelu_reducer,
                mxn_subtile_producer=hT_slice_producer,
                cache_tiles=True,
            )

            # ---- phase 2: yT[d, t] = sum_f w2[f, d] * hT[f, t] ----
            # w2 is the cached kxm (one DMA pass); hT is the SBUF-resident
            # kxn (no DMA). Gate + 1/128 descale applied on the host.
            kxm2_producer, kxm2_shape = dma_from_dram_kxm(kxm2_pool, w2_ap)
            kxn2_shape = ShapeInfo(pdims=((P, DFF // P),), fdims=(TPC,))

            def hT_kxn_producer(nc_, md):
                assert md.k_subtiles == 4
                return hT_tiles[md.k_tile_idx][:, :,
                                               ts(md.n_tile_idx, md.n_tile)]

            # Write each output group as one DMA per m-subtile, alternating
            # HWDGE engines, so the final (exposed) write lands ~4x faster
            # than a single 512 KB transfer on one queue.
            yT_t = yT_ap.rearrange("(po pi) f -> pi po f", pi=P)

            def yT_consumer(nc_, mxn_tile, md):
                for i in range(md.m_subtiles):
                    eng = (nc_.sync, nc_.scalar)[i % 2]
                    eng.dma_start(
                        yT_t[:, md.m_tile_idx * md.m_subtiles + i,
                             ts(md.n_tile_idx, md.n_tile)],
                        mxn_tile[:, i])

            composable_matmul_tile_kernel(
                tc,
                kxm_shape=kxm2_shape,
                kxn_shape=kxn2_shape,
                output_type=BF16,
                kxm_producer=kxm2_producer,
                kxn_producer=hT_kxn_producer,
                mxn_consumer=yT_consumer,
                cache_tiles=True,
                psum_n_bufs=2,
            )

    nc.compile()
    return nc


_NC = None


def _routing(hidden_states, router_weight, router_bias):
    """Same ops/backend as the reference => bit-identical selection."""
    import jax
    import jax.numpy as jnp
    scores = jnp.einsum('bsd,d->bs', hidden_states, router_weight) \
        + router_bias[0]
    top_scores, indices = jax.lax.top_k(scores, K)
    weights = jax.nn.sigmoid(top_scores)
    return np.asarray(indices), np.asarray(weights)


def _run(hidden_states, router_weight, router_bias, w1, b1, w2, b2,
         trace=False):
    global _NC
    hidden_states = np.asarray(hidden_states, dtype=np.float32)
    router_weight = np.asarray(router_weight, dtype=np.float32)
    router_bias = np.asarray(router_bias, dtype=np.float32)
    w1 = np.asarray(w1, dtype=np.float32)
    b1 = np.asarray(b1, dtype=np.float32)
    w2 = np.asarray(w2, dtype=np.float32)
    b2 = np.asarray(b2, dtype=np.float32)

    indices, weights = _routing(hidden_states, router_weight, router_bias)

    if _NC is None:
        _NC = _build_nc()

    w1_q = (w1 * np.float32(W1_SCALE)).astype(ml_dtypes.float8_e4m3fn)
    w2_q = (w2 * np.float32(W2_SCALE)).astype(ml_dtypes.float8_e4m3fn)
    b1v = np.ascontiguousarray(b1.reshape(DFF // P, P).T)

    in_maps = []
    core_idx = []  # (b, idx_slice) per core
    for c in range(N_CORES):
        b, h = divmod(c, 2)
        idx_c = indices[b, h * TPC:(h + 1) * TPC]
        gate_c = weights[b, h * TPC:(h + 1) * TPC]
        xT = hidden_states[b, idx_c].T.astype(ml_dtypes.float8_e4m3fn)
        in_maps.append({
            "xT": xT,
            "w1": w1_q,
            "w2": w2_q,
            "b1v": b1v,
        })
        core_idx.append((b, idx_c, gate_c))

    res = run_bass_kernel_spmd(_NC, in_maps, core_ids=list(range(N_CORES)),
                               trace=trace)

    out = hidden_states.copy().reshape(B * S, D)
    b2_nonzero = bool(np.any(b2))
    for c in range(N_CORES):
        b, idx_c, gate_c = core_idx[c]
        yT = res.results[c]["yT"].astype(np.float32)  # [D, TPC] bf16
        g = gate_c * np.float32(1.0 / W2_SCALE)
        y = yT.T * g[:, None]
        if b2_nonzero:
            y += gate_c[:, None] * b2[None, :]
        out[b * S + idx_c] += y
    return out.reshape(B, S, D), res


def kernel(**inputs):
    return _run(**inputs)[0]

